# revision 17
# baseline (speedup 1.0000x reference)
"""DIN (Deep Interest Network) attention kernel for Trainium2, 8-core data parallel.

Problem shapes (hardcoded, per spec):
  hist [8192, 200, 64] f32, target [8192, 64], other [8192, 128]
  aW1 [192, 96], ab1 [96], aW2 [96, 1], ab2 [1]
  oW1 [256, 128], ob1[128], oW2 [128, 85], ob2 [85], oW3 [85, 64], ob3 [64],
  fW [64, 1], fb [1]  ->  out [8192, 1] f32

Math per sample b (M=200 history steps, E=64):
  a = [hist, hist*tgt, tgt] [M, 192]
  h = relu(a @ aW1 + ab1); aij = h @ aW2 + ab2           (per m)
  tmp = sum_m aij[m] * hist[m]                            [64]
  x = [tmp, tgt, other]; 3x relu MLP; sigmoid -> scalar

Kernel strategy (per core, 1024 samples):
  - Fold the concat into per-sample weights: h = hist @ W_eff_b + bias_b with
    W_eff_b = aW1[0:64] + tgt_b * aW1[64:128], bias_b = tgt_b @ aW1[128:192] + ab1.
  - hist is DMA'd m-major in bf16 (cast-DMA), xbar-transposed to e-major
    pair-stacked tiles for the PE pass; the m-major bf16 copy is kept and used
    as the stationary operand of the final weighted-sum matmul.
  - aij rows are accumulated 32 samples at a time into one PSUM tile via
    one-hot-column weight tiles, then stream-transposed to m-major columns.
  - Output MLP runs in fp32 on 128-sample blocks.
"""

import numpy as np

import concourse.bass as bass
import concourse.mybir as mybir
import concourse.tile as tile

F32 = mybir.dt.float32
BF16 = mybir.dt.bfloat16
AOP = mybir.AluOpType
AFT = mybir.ActivationFunctionType

B, M, E, H, OTHER = 8192, 200, 64, 96, 128
NCORES = 8
BLOCK = 128            # samples per MLP block
AGRP = 32              # samples per aij accumulation group
G = 16                 # samples per hist DMA group
M1, M2 = 128, 72       # m chunks (200 = 128 + 72); chunk2 padded to 80 partitions
M2P = 80

INPUT_SPECS = [
    ("hist", (B, M, E)), ("target", (B, E)), ("other", (B, OTHER)),
    ("aW1", (192, 96)), ("ab1", (96,)), ("aW2", (96, 1)), ("ab2", (1,)),
    ("oW1", (256, 128)), ("ob1", (128,)), ("oW2", (128, 85)), ("ob2", (85,)),
    ("oW3", (85, 64)), ("ob3", (64,)), ("fW", (64, 1)), ("fb", (1,)),
]


def emit(tc, out_ap, ins, b_core):
    nc = tc.nc
    hist, target, other = ins["hist"], ins["target"], ins["other"]
    aW1, ab1, aW2, ab2 = ins["aW1"], ins["ab1"], ins["aW2"], ins["ab2"]
    oW1, ob1, oW2, ob2 = ins["oW1"], ins["ob1"], ins["oW2"], ins["ob2"]
    oW3, ob3, fW, fb = ins["oW3"], ins["ob3"], ins["fW"], ins["fb"]

    import contextlib
    ctx = contextlib.ExitStack()
    with ctx:
        const = ctx.enter_context(tc.tile_pool(name="const", bufs=1))
        # --- static weights / constants ---
        ident = const.tile([128, 128], F32, tag="ident")
        from concourse.masks import make_identity
        make_identity(nc, ident)

        # aW1 split, pair-replicated bf16 [128, 96]: rows 0:64 and 64:128 identical
        w1a = const.tile([128, 96], BF16, tag="w1a")
        w1b = const.tile([128, 96], BF16, tag="w1b")
        for half in (0, 64):
            nc.gpsimd.dma_start(out=w1a[half:half + 64, :], in_=aW1[0:64, :])
            nc.gpsimd.dma_start(out=w1b[half:half + 64, :], in_=aW1[64:128, :])
        w1c = const.tile([64, 96], F32, tag="w1c")
        nc.sync.dma_start(out=w1c, in_=aW1[128:192, :])
        ab1c = const.tile([96, 1], F32, tag="ab1c")
        nc.sync.dma_start(out=ab1c, in_=ab1)

        # aW2 one-hot-column tiles: z[:, j, :] is [96, 32] with aW2 in column j
        aw2b = const.tile([96, 1], BF16, tag="aw2b")
        nc.gpsimd.dma_start(out=aw2b, in_=aW2)
        zbuf = const.tile([96, AGRP, AGRP], BF16, tag="zbuf")
        nc.vector.memset(zbuf, 0.0)
        # one strided copy fills the (j, j) diagonal of all 32 one-hot tiles
        zdiag = bass.AP(tensor=zbuf.tensor, offset=zbuf.offset,
                        ap=[list(zbuf.ap[0]), [AGRP + 1, AGRP]])
        asrc = bass.AP(tensor=aw2b.tensor, offset=aw2b.offset,
                       ap=[list(aw2b.ap[0]), [0, AGRP]])
        nc.vector.tensor_copy(zdiag, asrc)
        ab2c = const.tile([AGRP, 1], F32, tag="ab2c")
        nc.sync.dma_start(out=ab2c, in_=bass.AP(
            tensor=ab2.tensor, offset=ab2.offset, ap=[[0, AGRP], [1, 1]]))

        # output MLP weights (fp32, natural layout = lhsT)
        ow1a = const.tile([128, 128], F32, tag="ow1a")
        ow1b = const.tile([128, 128], F32, tag="ow1b")
        nc.sync.dma_start(out=ow1a, in_=oW1[0:128, :])
        nc.sync.dma_start(out=ow1b, in_=oW1[128:256, :])
        ow2 = const.tile([128, 85], F32, tag="ow2")
        nc.sync.dma_start(out=ow2, in_=oW2)
        ow3 = const.tile([85, 64], F32, tag="ow3")
        nc.sync.dma_start(out=ow3, in_=oW3)
        fwt = const.tile([64, 1], F32, tag="fwt")
        nc.sync.dma_start(out=fwt, in_=fW)
        ob1c = const.tile([128, 1], F32, tag="ob1c")
        nc.sync.dma_start(out=ob1c, in_=ob1)
        ob2c = const.tile([85, 1], F32, tag="ob2c")
        nc.sync.dma_start(out=ob2c, in_=ob2)
        ob3c = const.tile([64, 1], F32, tag="ob3c")
        nc.sync.dma_start(out=ob3c, in_=ob3)
        fbc = const.tile([1, 1], F32, tag="fbc")
        nc.sync.dma_start(out=fbc, in_=fb)

        # --- pools ---
        blkp = ctx.enter_context(tc.tile_pool(name="blkp", bufs=2))
        histp = ctx.enter_context(tc.tile_pool(name="histp", bufs=4))
        tp = ctx.enter_context(tc.tile_pool(name="tp", bufs=3))
        aijp = ctx.enter_context(tc.tile_pool(name="aijp", bufs=2))
        psA = ctx.enter_context(tc.tile_pool(name="psA", bufs=2, space="PSUM"))
        psB = ctx.enter_context(tc.tile_pool(name="psB", bufs=2, space="PSUM"))
        psX = ctx.enter_context(tc.tile_pool(name="psX", bufs=2, space="PSUM"))

        nblocks = b_core // BLOCK
        for bi in range(nblocks):
            b0 = bi * BLOCK
            # ---- block prologue: target / other transposes, per-sample bias ----
            tgtn = blkp.tile([BLOCK, E], F32, tag="tgtn")
            nc.sync.dma_start(out=tgtn, in_=target[b0:b0 + BLOCK, :])
            othn = blkp.tile([BLOCK, OTHER], F32, tag="othn")
            nc.sync.dma_start(out=othn, in_=other[b0:b0 + BLOCK, :])

            pt = psX.tile([64, BLOCK], F32, tag="mlp")
            nc.tensor.transpose(pt, tgtn, ident)
            tgtT = blkp.tile([64, BLOCK], F32, tag="tgtT")
            nc.vector.tensor_copy(tgtT, pt)

            po = psX.tile([128, BLOCK], F32, tag="mlp")
            nc.tensor.transpose(po, othn, ident)
            xT1 = blkp.tile([128, BLOCK], F32, tag="xT1")
            nc.vector.tensor_copy(xT1, po)

            # bias_b^T = W1c.T @ tgt^T + ab1  -> [96, BLOCK] fp32
            pb = psX.tile([H, BLOCK], F32, tag="mlp")
            nc.tensor.matmul(pb, w1c, tgtT, start=True, stop=True)
            biasT = blkp.tile([H, BLOCK], F32, tag="biasT")
            nc.vector.tensor_scalar_add(biasT, pb, ab1c)

            # tgt^T in bf16, low and (DMA-shifted) high partition copies
            tgbL = blkp.tile([64, BLOCK], BF16, tag="tgbL")
            nc.vector.tensor_copy(tgbL, tgtT)
            tgbH = blkp.tile([128, BLOCK], BF16, tag="tgbH")
            nc.sync.dma_start(out=tgbH[64:128, :], in_=tgbL)
            # pair scalar columns: rows 0:64 = tgt of even sample, 64:128 = odd
            tpair = blkp.tile([128, BLOCK // 2], BF16, tag="tpair")
            ev = tgbL.rearrange("p (c two) -> p c two", two=2)
            od = tgbH[64:128, :].rearrange("p (c two) -> p c two", two=2)
            nc.vector.tensor_copy(tpair[0:64, :], ev[:, :, 0])
            nc.vector.tensor_copy(tpair[64:128, :], od[:, :, 1])

            # x^T chunk-0 tmp^T accumulator (one column per sample)
            xT0p = psB.tile([64, BLOCK], F32, tag="xT0p")

            for a in range(BLOCK // AGRP):        # 4 aij groups of 32
                aijps = psB.tile([AGRP, M], F32, tag="aijps")
                hts = []   # keep hist tiles of this group alive for tmp MMs
                for hg in range(AGRP // G):       # hist groups of G samples
                    s0 = b0 + a * AGRP + hg * G
                    hn1 = histp.tile([M1, G, E], BF16, tag="hn1")
                    src1 = hist[s0:s0 + G, 0:M1, :].rearrange("g m e -> m g e")
                    nc.gpsimd.dma_start(out=hn1, in_=src1)
                    hn2 = histp.tile([M2P, G, E], BF16, tag="hn2")
                    nc.vector.memset(hn2[64:M2P, :, :], 0.0)
                    src2 = hist[s0:s0 + G, M1:M, :].rearrange("g m e -> m g e")
                    nc.gpsimd.dma_start(out=hn2[0:M2, :, :], in_=src2)
                    hts.append((hn1, hn2))

                    for p in range(G // 2):       # pairs within hist group
                        jb = a * AGRP + hg * G + 2 * p   # block-local sample
                        # e-major pair-stacked hist^T [128, 208]
                        hT = tp.tile([128, M1 + M2P], BF16, tag="hT")
                        nc.sync.dma_start_transpose(
                            hT[:, 0:M1], hn1[:, 2 * p:2 * p + 2, :])
                        nc.sync.dma_start_transpose(
                            hT[:, M1:M1 + M2P], hn2[:, 2 * p:2 * p + 2, :])
                        # W_eff pair [128, 96] = W1a + tgt*W1b (both halves)
                        wef = tp.tile([128, H], BF16, tag="wef")
                        nc.vector.scalar_tensor_tensor(
                            out=wef, in0=w1b, scalar=tpair[:, jb // 2:jb // 2 + 1],
                            in1=w1a, op0=AOP.mult, op1=AOP.add)
                        for s in range(2):
                            js = jb + s
                            hps = psA.tile([H, M], F32, tag="hps")
                            nc.tensor.matmul(
                                hps, wef[64 * s:64 * s + 64, :],
                                hT[64 * s:64 * s + 64, 0:M],
                                start=True, stop=True)
                            hsb = tp.tile([H, M], BF16, tag="hsb")
                            if js % 2 == 0:
                                nc.vector.tensor_scalar(
                                    out=hsb, in0=hps,
                                    scalar1=biasT[:, js:js + 1], scalar2=0.0,
                                    op0=AOP.add, op1=AOP.max)
                            else:
                                nc.scalar.activation(
                                    hsb, hps, AFT.Relu,
                                    bias=biasT[:, js:js + 1], scale=1.0)
                            j = js - a * AGRP
                            nc.tensor.matmul(
                                aijps, zbuf[:, j, :], hsb,
                                start=(j == 0), stop=(j == AGRP - 1),
                                skip_group_check=True)

                # aij rows -> SBUF (+ab2, cast bf16), pad cols for transpose
                aijs = aijp.tile([AGRP, 224], BF16, tag="aijs")
                nc.vector.memset(aijs[:, M:224], 0.0)
                nc.vector.tensor_scalar_add(aijs[:, 0:M], aijps, ab2c)
                aT1 = aijp.tile([128, AGRP], BF16, tag="aT1")
                aT2 = aijp.tile([96, AGRP], BF16, tag="aT2")
                for blk in range(7):
                    dst = aT1 if blk < 4 else aT2
                    r0 = 32 * blk if blk < 4 else 32 * (blk - 4)
                    nc.vector.transpose(
                        dst[r0:r0 + 32, :], aijs[:, 32 * blk:32 * (blk + 1)])

                # tmp^T columns via hist-stationary matmuls
                for hg in range(AGRP // G):
                    hn1, hn2 = hts[hg]
                    for gi in range(G):
                        j = hg * G + gi
                        col = a * AGRP + j
                        nc.tensor.matmul(
                            xT0p[0:64, col:col + 1], hn1[:, gi, :],
                            aT1[:, j:j + 1],
                            start=True, stop=False, skip_group_check=True)
                        nc.tensor.matmul(
                            xT0p[0:64, col:col + 1], hn2[0:M2, gi, :],
                            aT2[0:M2, j:j + 1],
                            start=False, stop=True, skip_group_check=True)

            # ---- block epilogue: output MLP on 128 samples ----
            xT0 = blkp.tile([128, BLOCK], F32, tag="xT0")
            nc.vector.tensor_copy(xT0[0:64, :], xT0p)
            nc.sync.dma_start(out=xT0[64:128, :], in_=tgtT)
            p1 = psX.tile([128, BLOCK], F32, tag="mlp")
            nc.tensor.matmul(p1, ow1a, xT0, start=True, stop=False)
            nc.tensor.matmul(p1, ow1b, xT1, start=False, stop=True)
            s1 = blkp.tile([128, BLOCK], F32, tag="s1")
            nc.scalar.activation(s1, p1, AFT.Relu, bias=ob1c, scale=1.0)
            p2 = psX.tile([85, BLOCK], F32, tag="mlp")
            nc.tensor.matmul(p2, ow2, s1, start=True, stop=True)
            s2 = blkp.tile([85, BLOCK], F32, tag="s2")
            nc.scalar.activation(s2, p2, AFT.Relu, bias=ob2c, scale=1.0)
            p3 = psX.tile([64, BLOCK], F32, tag="mlp")
            nc.tensor.matmul(p3, ow3, s2, start=True, stop=True)
            s3 = blkp.tile([64, BLOCK], F32, tag="s3")
            nc.scalar.activation(s3, p3, AFT.Relu, bias=ob3c, scale=1.0)
            p4 = psX.tile([1, BLOCK], F32, tag="mlp")
            nc.tensor.matmul(p4, fwt, s3, start=True, stop=True)
            ob = blkp.tile([1, BLOCK], F32, tag="ob")
            nc.scalar.activation(ob, p4, AFT.Sigmoid, bias=fbc, scale=1.0)
            nc.sync.dma_start(out=out_ap[b0:b0 + BLOCK, :], in_=ob)


def build(b_core):
    from concourse import bacc
    nc = bacc.Bacc("TRN2", target_bir_lowering=False, debug=False)
    ins = {}
    for name, shape in INPUT_SPECS:
        shard = list(shape)
        if name in ("hist", "target", "other"):
            shard[0] = b_core
        t = nc.dram_tensor(name, shard, F32, kind="ExternalInput")
        ap = t.ap()
        if len(shard) == 1:
            ap = ap.rearrange("(a one) -> a one", one=1)
        ins[name] = ap
    out_t = nc.dram_tensor("out", [b_core, 1], F32, kind="ExternalOutput")
    with tile.TileContext(nc) as tc:
        emit(tc, out_t.ap(), ins, b_core)
    nc.compile()
    return nc


def kernel(**inputs):
    b_core = B // NCORES
    nc = build(b_core)
    in_maps = []
    for c in range(NCORES):
        m = {}
        for name, _ in INPUT_SPECS:
            arr = np.ascontiguousarray(np.asarray(inputs[name], dtype=np.float32))
            if name in ("hist", "target", "other"):
                arr = arr[c * b_core:(c + 1) * b_core]
            m[name] = arr
        in_maps.append(m)
    from concourse.bass_utils import run_bass_kernel_spmd
    res = run_bass_kernel_spmd(nc, in_maps, core_ids=list(range(NCORES)))
    return np.concatenate([r["out"] for r in res.results], axis=0)
